# revision 1
# baseline (speedup 1.0000x reference)
"""Trainium2 Bass kernel for nn_MemoryBank (vq_codebook softmax).

C[b, s, t] = softmax_s(-||H[b,:,t] - units[:,s]||^2)
           = softmax_s(2*cross[s,t] - m_sq[s]),  cross = units.T @ H[b]

Strategy (8 NeuronCores, data-parallel over batch B=64 -> 8 per core):
  - bf16 3-term split GEMM (h1u1 + h1u2 + h2u1) for ~fp32-accurate logits,
    with -m_sq/2 folded in as a K=2 augmentation row (bf16 hi+lo split).
  - Layout: s on partitions (output-native), t blocks of 256 on free axis.
    Cross accumulates into PSUM, two s-blocks packed per 512-col bank.
  - Softmax without any cross-partition sum machinery:
      max:  DVE max-chain over the 4 banks + GPSIMD partition_all_reduce
            -> rank-1 matmul adds -max/2 into PSUM (exact cancellation,
            bf16 rounding of the shift is harmless).
      exp1: ACT Exp(scale=2) PSUM -> fp16 SBUF tiles.
      den:  PE ones-matmul over exp1 -> [1, 512] PSUM per bank.
      out:  ACT Ln of den; -ln(den)/2 split into bf16 hi+lo -> K=2 rank-1
            matmul into PSUM; second ACT Exp(scale=2) emits the final
            normalized probabilities directly (fp32), DMA out.
"""
import numpy as np
import ml_dtypes

import concourse.bacc as bacc
import concourse.bass as bass
import concourse.bass_isa as bass_isa
import concourse.mybir as mybir
import concourse.tile as tile
from concourse.tile import add_dep_helper

F32 = mybir.dt.float32
BF16 = mybir.dt.bfloat16
FP16 = mybir.dt.float16
AF = mybir.ActivationFunctionType
ALU = mybir.AluOpType

# Problem shape (hardcoded per harness contract)
B, D, T, S = 64, 512, 2048, 1024
NCORES = 8
B_SH = B // NCORES          # batches per core
DC = D // 128               # d chunks of 128
SBK = S // 128              # s blocks of 128 (partition dim of output)
TBL = 256                   # t block (free axis) per region
NBK = 4                     # cross banks per group (2 s-blocks each)


def build_kernel(b_sh=B_SH, t=T, tbl=TBL):
    ntb = t // tbl
    nc = bacc.Bacc(None, target_bir_lowering=False, debug=False)

    h1_d = nc.dram_tensor("h1", [b_sh, DC, 128, t], BF16, kind="ExternalInput")
    h2_d = nc.dram_tensor("h2", [b_sh, DC, 128, t], BF16, kind="ExternalInput")
    u1_d = nc.dram_tensor("u1", [DC, 128, S], BF16, kind="ExternalInput")
    u2_d = nc.dram_tensor("u2", [DC, 128, S], BF16, kind="ExternalInput")
    um_d = nc.dram_tensor("um", [2, S], BF16, kind="ExternalInput")
    c_d = nc.dram_tensor("C", [b_sh, S, t], F32, kind="ExternalOutput")

    w2 = 2 * tbl                # bank width (512 cols fp32)

    with tile.TileContext(nc) as tc:
        with (
            tc.tile_pool(name="const", bufs=1) as cpool,
            tc.tile_pool(name="hbuf", bufs=2) as hpool,
            tc.tile_pool(name="work", bufs=3) as wpool,
            tc.tile_pool(name="expp", bufs=3) as epool,
            tc.tile_pool(name="outp", bufs=3) as opool,
            tc.tile_pool(name="ps", bufs=2, space="PSUM") as ps,
            tc.tile_pool(name="pstat", bufs=1, space="PSUM") as pstat,
        ):
            # --- constants loaded once ---
            u1_sb = cpool.tile([128, DC, S], BF16, tag="u1")
            u2_sb = cpool.tile([128, DC, S], BF16, tag="u2")
            nc.sync.dma_start(u1_sb[:], u1_d.rearrange("c p s -> p c s"))
            nc.sync.dma_start(u2_sb[:], u2_d.rearrange("c p s -> p c s"))
            um_sb = cpool.tile([2, S], BF16, tag="um")
            nc.sync.dma_start(um_sb[:], um_d[:])
            ones2 = cpool.tile([2, tbl], BF16, tag="ones2")
            nc.vector.memset(ones2[:], 1.0)
            ones_1x128 = cpool.tile([1, 128], BF16, tag="ones_1x128")
            nc.vector.memset(ones_1x128[:], 1.0)
            ones_2x128 = cpool.tile([2, 128], BF16, tag="ones_2x128")
            nc.vector.memset(ones_2x128[:], 1.0)
            ones_128x1 = cpool.tile([128, 1], FP16, tag="ones_128x1")
            nc.vector.memset(ones_128x1[:], 1.0)

            for b in range(b_sh):
                h1_sb = hpool.tile([128, DC, t], BF16, tag="h1")
                h2_sb = hpool.tile([128, DC, t], BF16, tag="h2")
                nc.sync.dma_start(h1_sb[:], h1_d[b].rearrange("c p t -> p c t"))
                nc.sync.dma_start(h2_sb[:], h2_d[b].rearrange("c p t -> p c t"))

                for it in range(ntb):
                    t0 = it * tbl
                    # --- cross: 4 banks, 2 s-blocks per bank ---
                    # bank k holds s-blocks 2k (cols 0:tbl) and 2k+1 (tbl:2tbl)
                    cr = [ps.tile([128, w2], F32, tag=f"cr{k}", name=f"cr{k}",
                                  bufs=(2 if k < 3 else 1)) for k in range(NBK)]

                    for k in range(NBK):
                        for half in range(2):
                            sb = 2 * k + half
                            s0 = sb * 128
                            reg = cr[k][:, half * tbl:(half + 1) * tbl]
                            # aug row: -m_sq/2 (bf16 hi+lo). start=True only
                            # on the bank's first matmul: it marks the WHOLE
                            # 2KB zero-region pending-zero; later matmuls
                            # lazily zero-then-write their own bytes.
                            nc.tensor.matmul(
                                reg, um_sb[:, s0:s0 + 128], ones2[:],
                                start=(half == 0), stop=False,
                                skip_group_check=True,
                            )
                            for c in range(DC):
                                for i, (uu, hh) in enumerate(
                                    ((u1_sb, h1_sb), (u1_sb, h2_sb),
                                     (u2_sb, h1_sb))
                                ):
                                    nc.tensor.matmul(
                                        reg,
                                        uu[:, c, s0:s0 + 128],
                                        hh[:, c, t0:t0 + tbl],
                                        start=False, stop=False,
                                        skip_group_check=True,
                                    )

                    # --- max over s: DVE chain over banks, fold halves ---
                    acc = wpool.tile([128, w2], F32, tag="acc")
                    nc.vector.tensor_copy(acc[:], cr[0][:])
                    for k in range(1, NBK):
                        nc.vector.tensor_max(acc[:], acc[:], cr[k][:])
                    tmax = wpool.tile([128, tbl], F32, tag="tmax")
                    nc.vector.tensor_max(
                        tmax[:], acc[:, 0:tbl], acc[:, tbl:2 * tbl])
                    mbc = wpool.tile([128, tbl], F32, tag="mbc")
                    nc.gpsimd.partition_all_reduce(
                        mbc[:], tmax[:], channels=128,
                        reduce_op=bass_isa.ReduceOp.max,
                    )
                    # PSUM holds l/2, so its max is M/2 already; the rank-1
                    # shift must subtract exactly mbc (scale=2 at exp time).
                    mh = wpool.tile([1, tbl], BF16, tag="mh")
                    nc.vector.tensor_scalar_mul(mh[0:1, :], mbc[0:1, :], -1.0)

                    # --- rank-1 shift (per region) + exp1 (fp16, per bank) ---
                    exps = []
                    for k in range(NBK):
                        last_r1 = None
                        for half in range(2):
                            last_r1 = nc.tensor.matmul(
                                cr[k][:, half * tbl:(half + 1) * tbl],
                                ones_1x128[:], mh[:],
                                start=False, stop=False, skip_group_check=True,
                            )
                        ex = epool.tile([128, w2], FP16, tag=f"ex{k}",
                                        name=f"ex{k}")
                        e1 = nc.scalar.activation(
                            ex[:], cr[k][:], AF.Exp, scale=2.0)
                        # Tile's accumulate-group tracking misses the RAW dep
                        # on the second rank-1 matmul; add it explicitly.
                        add_dep_helper(e1.ins, last_r1.ins, sync=True,
                                       reason="exp1 after rank1 shift")
                        exps.append(ex)

                    # --- denominator: PE ones-matmul -> [1, 512] accumulated
                    #     over banks; fold the two half-columns -> [1, 256] ---
                    den = pstat.tile([1, w2], F32, tag="den")
                    for k in range(NBK):
                        nc.tensor.matmul(
                            den[:], ones_128x1[:], exps[k][:],
                            start=(k == 0), stop=(k == NBK - 1),
                        )
                    dcp = wpool.tile([1, w2], F32, tag="dcp")
                    nc.vector.tensor_copy(dcp[:], den[:])
                    dsum = wpool.tile([1, tbl], F32, tag="dsum")
                    nc.vector.tensor_add(
                        dsum[:], dcp[0:1, 0:tbl], dcp[0:1, tbl:2 * tbl])
                    lnden = wpool.tile([1, tbl], F32, tag="lnden")
                    nc.scalar.activation(lnden[:], dsum[:], AF.Ln)
                    # split -ln(den)/2 into bf16 hi+lo rows [1, 256] each
                    ln_hi = wpool.tile([1, tbl], BF16, tag="ln_hi")
                    ln_lo = wpool.tile([1, tbl], BF16, tag="ln_lo")
                    lnlo_f = wpool.tile([1, tbl], F32, tag="lnlo_f")
                    nc.vector.tensor_scalar_mul(ln_hi[:], lnden[:], -0.5)
                    nc.vector.scalar_tensor_tensor(
                        lnlo_f[:], lnden[:], -0.5, ln_hi[:],
                        op0=ALU.mult, op1=ALU.subtract,
                    )
                    nc.vector.tensor_copy(ln_lo[:], lnlo_f[:])

                    # --- rank-1 -ln(den)/2 (per region) + exp2 -> output ---
                    for k in range(NBK):
                        last_r2 = None
                        for half in range(2):
                            reg = cr[k][:, half * tbl:(half + 1) * tbl]
                            nc.tensor.matmul(
                                reg, ones_1x128[:], ln_hi[:],
                                start=False, stop=False, skip_group_check=True,
                            )
                            last_r2 = nc.tensor.matmul(
                                reg, ones_1x128[:], ln_lo[:],
                                start=False, stop=(half == 1),
                                skip_group_check=True,
                            )
                        ot = opool.tile([128, w2], F32, tag=f"ot{k}",
                                        name=f"ot{k}")
                        e2 = nc.scalar.activation(
                            ot[:], cr[k][:], AF.Exp, scale=2.0)
                        add_dep_helper(e2.ins, last_r2.ins, sync=True,
                                       reason="exp2 after rank2 lnden")
                        for half in range(2):
                            sb = 2 * k + half
                            nc.sync.dma_start(
                                c_d[b, sb * 128:(sb + 1) * 128, t0:t0 + tbl],
                                ot[:, half * tbl:(half + 1) * tbl],
                            )

    nc.compile()
    return nc


# ---------------------------------------------------------------- host side

_RUNNER = None


def _get_runner():
    global _RUNNER
    if _RUNNER is None:
        nc = build_kernel()
        _RUNNER = _BassPjrtRunner(nc, NCORES)
    return _RUNNER


def _split_bf16(x):
    hi = x.astype(ml_dtypes.bfloat16)
    lo = (x - hi.astype(np.float32)).astype(ml_dtypes.bfloat16)
    return hi, lo


def prep_inputs(H, units):
    H = np.ascontiguousarray(np.asarray(H, dtype=np.float32))
    U = np.ascontiguousarray(np.asarray(units, dtype=np.float32))
    h1, h2 = _split_bf16(H)
    u1, u2 = _split_bf16(U)
    msq_half = -(U.astype(np.float64) ** 2).sum(0).astype(np.float32) * 0.5
    m1 = msq_half.astype(ml_dtypes.bfloat16)
    m2 = (msq_half - m1.astype(np.float32)).astype(ml_dtypes.bfloat16)
    um = np.stack([m1, m2], 0)

    u1 = u1.reshape(DC, 128, S)
    u2 = u2.reshape(DC, 128, S)
    in_maps = []
    for c in range(NCORES):
        sl = slice(c * B_SH, (c + 1) * B_SH)
        in_maps.append({
            "h1": h1[sl].reshape(B_SH, DC, 128, T),
            "h2": h2[sl].reshape(B_SH, DC, 128, T),
            "u1": u1, "u2": u2, "um": um,
        })
    return in_maps


def kernel(H, units):
    runner = _get_runner()
    in_maps = prep_inputs(H, units)
    args = runner.prep_inputs(in_maps)
    outs = runner.run(args)
    c = np.asarray(outs[0])           # (NCORES*B_SH, S, T) concat on axis 0
    return c.reshape(B, S, T)


# ------------------------------------------------- embedded PJRT runner

class _BassPjrtRunner:
    def __init__(self, nc, n_cores):
        import jax
        from jax.sharding import Mesh, PartitionSpec
        from jax.experimental.shard_map import shard_map
        from concourse import bass2jax

        bass2jax.install_neuronx_cc_hook()
        self.n_cores = n_cores
        partition_name = (
            nc.partition_id_tensor.name if nc.partition_id_tensor else None
        )
        in_names, out_names, out_avals, zero_outs = [], [], [], []
        for alloc in nc.m.functions[0].allocations:
            if not isinstance(alloc, mybir.MemoryLocationSet):
                continue
            name = alloc.memorylocations[0].name
            if alloc.kind == "ExternalInput":
                if name != partition_name:
                    in_names.append(name)
            elif alloc.kind == "ExternalOutput":
                shape = tuple(alloc.tensor_shape)
                dtype = mybir.dt.np(alloc.dtype)
                out_names.append(name)
                out_avals.append(jax.core.ShapedArray(shape, dtype))
                zero_outs.append((shape, dtype))
        self.in_names = in_names
        self.out_names = out_names
        self.out_shapes = zero_outs
        n_params = len(in_names)
        n_outs = len(out_avals)
        all_in_names = in_names + out_names
        if partition_name is not None:
            all_in_names.append(partition_name)
        self.n_params = n_params

        def _body(*args):
            operands = list(args)
            if partition_name is not None:
                operands.append(bass2jax.partition_id_tensor())
            outs = bass2jax._bass_exec_p.bind(
                *operands,
                out_avals=tuple(out_avals),
                in_names=tuple(all_in_names),
                out_names=tuple(out_names),
                lowering_input_output_aliases=(),
                sim_require_finite=False,
                sim_require_nnan=False,
                nc=nc,
            )
            return tuple(outs)

        devices = jax.devices()[:n_cores]
        assert len(devices) == n_cores
        if n_cores == 1:
            self._fn = jax.jit(_body, keep_unused=True)
        else:
            mesh = Mesh(np.asarray(devices), ("core",))
            in_specs = (PartitionSpec("core"),) * (n_params + n_outs)
            out_specs = (PartitionSpec("core"),) * n_outs
            self._fn = jax.jit(
                shard_map(_body, mesh=mesh, in_specs=in_specs,
                          out_specs=out_specs, check_rep=False),
                keep_unused=True,
            )

    def prep_inputs(self, in_maps):
        per_core = [[np.asarray(m[n]) for n in self.in_names] for m in in_maps]
        if self.n_cores == 1:
            args = per_core[0]
        else:
            args = [
                np.concatenate([per_core[c][i] for c in range(self.n_cores)], 0)
                for i in range(self.n_params)
            ]
        zouts = []
        for (s, d) in self.out_shapes:
            full = (s[0] * self.n_cores,) + tuple(s[1:]) \
                if self.n_cores > 1 else s
            zouts.append(np.zeros(full, d))
        return args + zouts

    def run(self, args):
        import jax
        outs = self._fn(*args)
        jax.block_until_ready(outs)
        return outs



# revision 15
# speedup vs baseline: 14221.8049x; 14221.8049x over previous
"""Trainium2 Bass kernel for nn_MemoryBank (vq_codebook softmax).

C[b, s, t] = softmax_s(-||H[b,:,t] - units[:,s]||^2)
           = softmax_s(2*cross[s,t] - m_sq[s]),  cross = units.T @ H[b]

Strategy (8 NeuronCores, data-parallel over batch B=64 -> 8 per core):
  - bf16 3-term split GEMM (h1u1 + h1u2 + h2u1) for ~fp32-accurate logits
    (fp32r 1-pass was measured at 2.4e-2 softmax max-rel -- fails the gate).
  - Layout: s on partitions (output-native), t blocks of 256 on free axis.
    Cross accumulates into PSUM, two s-blocks packed per 512-col bank.
  - Per t-block softmax over s:
      max:  DVE scalar_tensor_tensor chain ((cr - msq/2) max acc) over the
            8 half-banks + GPSIMD partition_all_reduce -> one K=1 bf16
            rank-1 matmul per bank adds -max/2 into PSUM (cancels exactly;
            bf16 rounding of the shift is harmless).
      exp1: ACT Exp(scale=2, bias=-msq fp32 per-partition AP) -> bf16 SBUF.
            Only ACT function used -> no ACT table reloads.
      den:  PE ones-matmul over exp1 -> [1, 512] PSUM; fold halves; DVE
            reciprocal; PE rank-1 broadcasts the [1,256] recip row into a
            PSUM bank.
      out:  DVE multiply exp1 * recip_bcast -> fp32 SBUF -> DMA.
  - The emission is software-pipelined 3 deep (cross(j) | max/exp1/den(j-1)
    | recip/out(j-2)) so the PE FIFO never waits on the GPSIMD/DVE chain;
    PE gaps stay under the ~3.4us HAM window and the PE runs warm (2.4 GHz)
    throughout (unpipelined: 94 HAM throttle episodes, 1.36 ms; pipelined:
    0.91 ms, PE 91% busy).
"""
import numpy as np
import ml_dtypes

import concourse.bacc as bacc
import concourse.bass as bass
import concourse.bass_isa as bass_isa
import concourse.mybir as mybir
import concourse.tile as tile
from concourse.tile import add_dep_helper

F32 = mybir.dt.float32
BF16 = mybir.dt.bfloat16
FP16 = mybir.dt.float16
AF = mybir.ActivationFunctionType
ALU = mybir.AluOpType

# Problem shape (hardcoded per harness contract)
B, D, T, S = 64, 512, 2048, 1024
NCORES = 8
B_SH = B // NCORES          # batches per core
DC = D // 128               # d chunks of 128
SBK = S // 128              # s blocks of 128 (partition dim of output)
TBL = 256                   # t block (free axis) per region
NBK = 4                     # cross banks per group (2 s-blocks each)


def build_kernel(b_sh=B_SH, t=T, tbl=TBL):
    ntb = t // tbl
    nc = bacc.Bacc(None, target_bir_lowering=False, debug=False)

    h1_d = nc.dram_tensor("h1", [b_sh, DC, 128, t], BF16, kind="ExternalInput")
    h2_d = nc.dram_tensor("h2", [b_sh, DC, 128, t], BF16, kind="ExternalInput")
    u1_d = nc.dram_tensor("u1", [DC, 128, S], BF16, kind="ExternalInput")
    u2_d = nc.dram_tensor("u2", [DC, 128, S], BF16, kind="ExternalInput")
    mb_d = nc.dram_tensor("mb", [2, SBK, 128], F32, kind="ExternalInput")
    c_d = nc.dram_tensor("C", [b_sh, S, t], F32, kind="ExternalOutput")

    w2 = 2 * tbl
    steps = [(b, it) for b in range(b_sh) for it in range(ntb)]
    n = len(steps)

    with tile.TileContext(nc) as tc:
        with (
            tc.tile_pool(name="const", bufs=1) as cpool,
            tc.tile_pool(name="hbuf", bufs=2) as hpool,
            tc.tile_pool(name="work", bufs=3) as wpool,
            tc.tile_pool(name="expp", bufs=3) as epool,
            tc.tile_pool(name="outp", bufs=3) as opool,
            tc.tile_pool(name="ps", bufs=2, space="PSUM") as ps,
            tc.tile_pool(name="pstat", bufs=1, space="PSUM") as pstat,
        ):
            u1_sb = cpool.tile([128, DC, S], BF16, tag="u1")
            u2_sb = cpool.tile([128, DC, S], BF16, tag="u2")
            nc.sync.dma_start(u1_sb[:], u1_d.rearrange("c p s -> p c s"))
            nc.sync.dma_start(u2_sb[:], u2_d.rearrange("c p s -> p c s"))
            mb_sb = cpool.tile([128, 2, SBK], F32, tag="mb")
            nc.sync.dma_start(mb_sb[:], mb_d.rearrange("r k p -> p r k"))
            ones_1x128 = cpool.tile([1, 128], BF16, tag="ones_1x128")
            nc.vector.memset(ones_1x128[:], 1.0)
            ones_1x128f = cpool.tile([1, 128], FP16, tag="ones_1x128f")
            nc.vector.memset(ones_1x128f[:], 1.0)
            ones_128x1 = cpool.tile([128, 1], BF16, tag="ones_128x1")
            nc.vector.memset(ones_128x1[:], 1.0)

            hbufs = {}

            def load_h(b):
                # chunked along t so the first cross of a batch only waits
                # on its own t-slice, not the full 4MB batch load
                h1_sb = hpool.tile([128, DC, t], BF16, tag="h1")
                h2_sb = hpool.tile([128, DC, t], BF16, tag="h2")
                h1v = h1_d[b].rearrange("c p t -> p c t")
                h2v = h2_d[b].rearrange("c p t -> p c t")
                for it in range(ntb):
                    t0 = it * tbl
                    nc.sync.dma_start(h1_sb[:, :, t0:t0 + tbl],
                                      h1v[:, :, t0:t0 + tbl])
                    nc.sync.dma_start(h2_sb[:, :, t0:t0 + tbl],
                                      h2v[:, :, t0:t0 + tbl])
                hbufs[b] = (h1_sb, h2_sb)

            load_h(0)
            st = [None] * n    # per-step pipeline state

            def cross_banks(j, banks, holdback=False):
                # holdback: skip the last 2 chunks of the last bank's second
                # half; they are emitted later (PE filler between the rank-1
                # block and cross banks 2,3, giving ACT exp1 slack to finish
                # before the single-buffered banks are rewritten)
                b, it = steps[j]
                t0 = it * tbl
                h1_sb, h2_sb = hbufs[b]
                d = st[j]
                for k in banks:
                    for half in range(2):
                        sb = 2 * k + half
                        s0 = sb * 128
                        reg = d["cr"][k][:, half * tbl:(half + 1) * tbl]
                        for c in range(DC):
                            if (holdback and k == banks[-1] and half == 1
                                    and c >= DC - 2):
                                continue
                            for i, (uu, hh) in enumerate(
                                ((u1_sb, h1_sb), (u1_sb, h2_sb),
                                 (u2_sb, h1_sb))
                            ):
                                nc.tensor.matmul(
                                    reg,
                                    uu[:, c, s0:s0 + 128],
                                    hh[:, c, t0:t0 + tbl],
                                    start=(half == 0 and c == 0 and i == 0),
                                    stop=False,
                                    skip_group_check=True,
                                )

            def cross_rest(j, k):
                # the held-back MMs of bank k's second half
                b, it = steps[j]
                t0 = it * tbl
                h1_sb, h2_sb = hbufs[b]
                d = st[j]
                sb = 2 * k + 1
                s0 = sb * 128
                reg = d["cr"][k][:, tbl:2 * tbl]
                for c in range(DC - 2, DC):
                    for (uu, hh) in ((u1_sb, h1_sb), (u1_sb, h2_sb),
                                     (u2_sb, h1_sb)):
                        nc.tensor.matmul(
                            reg, uu[:, c, s0:s0 + 128], hh[:, c, t0:t0 + tbl],
                            start=False, stop=False, skip_group_check=True,
                        )

            for j in range(n + 2):
                A = j if j < n else None            # cross stage
                Bst = j - 1 if 1 <= j <= n else None    # max/exp1/den stage
                Cst = j - 2 if 2 <= j <= n + 1 else None  # rc_bc/out stage

                # --- 1. max chain (DVE) + partition reduce (GPSIMD), step B ---
                if Bst is not None:
                    d = st[Bst]
                    acc = wpool.tile([128, tbl], F32, tag="acc")
                    first = True
                    for k in range(NBK):
                        for half in range(2):
                            sb = 2 * k + half
                            src = d["cr"][k][:, half * tbl:(half + 1) * tbl]
                            msq2 = mb_sb[:, 1, sb:sb + 1]
                            if first:
                                nc.vector.tensor_scalar(
                                    acc[:], src, msq2, None, op0=ALU.add)
                                first = False
                            else:
                                nc.vector.scalar_tensor_tensor(
                                    acc[:], src, msq2, acc[:],
                                    op0=ALU.add, op1=ALU.max)
                    mbc = wpool.tile([128, tbl], F32, tag="mbc")
                    nc.gpsimd.partition_all_reduce(
                        mbc[:], acc[:], channels=128,
                        reduce_op=bass_isa.ReduceOp.max,
                    )
                    d["mbc"] = mbc

                # --- 2. cross banks 0,1 for step A (+ h prefetch) ---
                if A is not None:
                    b, it = steps[A]
                    st[A] = {"cr": [ps.tile([128, w2], F32, tag=f"cr{k}",
                                            name=f"cr{k}",
                                            bufs=(2 if k < 2 else 1))
                                    for k in range(NBK)]}
                    cross_banks(A, (0, 1), holdback=True)
                    if it == max(ntb - 2, 0) and b + 1 < b_sh:
                        load_h(b + 1)

                # --- 3. den fold + reciprocal row, step C ---
                if Cst is not None:
                    d = st[Cst]
                    dcp = wpool.tile([1, w2], F32, tag="dcp")
                    nc.vector.tensor_copy(dcp[:], d["den"][:])
                    dsum = wpool.tile([1, tbl], F32, tag="dsum")
                    nc.vector.tensor_add(
                        dsum[:], dcp[0:1, 0:tbl], dcp[0:1, tbl:2 * tbl])
                    rcpf = wpool.tile([1, tbl], F32, tag="rcpf")
                    nc.vector.reciprocal(rcpf[:], dsum[:])
                    rc_row = wpool.tile([1, w2], FP16, tag="rc_row")
                    nc.vector.tensor_copy(rc_row[0:1, 0:tbl], rcpf[:])
                    nc.vector.tensor_copy(rc_row[0:1, tbl:w2], rcpf[:])
                    d["rc_row"] = rc_row

                # --- 4. mh row (-max/2, both halves), step B ---
                if Bst is not None:
                    d = st[Bst]
                    mh = wpool.tile([1, w2], BF16, tag="mh")
                    nc.vector.tensor_scalar_mul(
                        mh[0:1, 0:tbl], d["mbc"][0:1, :], -1.0)
                    nc.vector.tensor_scalar_mul(
                        mh[0:1, tbl:w2], d["mbc"][0:1, :], -1.0)
                    d["mh"] = mh

                # --- 5. rank-1 -max/2 + exp1 (bank order 2,3,0,1), step B ---
                if Bst is not None:
                    d = st[Bst]
                    d["ex"] = [None] * NBK
                    for k in (2, 3, 0, 1):
                        r1 = nc.tensor.matmul(
                            d["cr"][k][:], ones_1x128[:], d["mh"][:],
                            start=False, stop=True, skip_group_check=True,
                        )
                        ex = epool.tile([128, w2], BF16, tag=f"ex{k}",
                                        name=f"ex{k}")
                        for half in range(2):
                            sb = 2 * k + half
                            e1 = nc.scalar.activation(
                                ex[:, half * tbl:(half + 1) * tbl],
                                d["cr"][k][:, half * tbl:(half + 1) * tbl],
                                AF.Exp, bias=mb_sb[:, 0, sb:sb + 1], scale=2.0)
                            add_dep_helper(e1.ins, r1.ins, sync=True,
                                           reason="exp1 after rank1 shift")
                        d["ex"][k] = ex

                # --- 5b. held-back bank-1 cross MMs (PE filler), step A ---
                if A is not None:
                    cross_rest(A, 1)

                # --- 6. broadcast recip across partitions (PE), step C ---
                if Cst is not None:
                    d = st[Cst]
                    rc_bc = pstat.tile([128, w2], F32, tag="bc")
                    nc.tensor.matmul(rc_bc[:], ones_1x128f[:],
                                     d["rc_row"][:], start=True, stop=True)
                    d["rc_bc"] = rc_bc

                # --- 7. cross banks 2,3 for step A ---
                if A is not None:
                    cross_banks(A, (2, 3))

                # --- 8. out = exp1 * recip (DVE) + DMA, step C ---
                if Cst is not None:
                    d = st[Cst]
                    b, it = steps[Cst]
                    t0 = it * tbl
                    for k in range(NBK):
                        ot = opool.tile([128, w2], F32, tag=f"ot{k}",
                                        name=f"ot{k}")
                        nc.vector.tensor_mul(ot[:], d["ex"][k][:],
                                             d["rc_bc"][:])
                        for half in range(2):
                            sb = 2 * k + half
                            nc.sync.dma_start(
                                c_d[b, sb * 128:(sb + 1) * 128, t0:t0 + tbl],
                                ot[:, half * tbl:(half + 1) * tbl],
                            )
                    st[Cst] = None

                # --- 9. denominator (PE ones-matmul), step B ---
                if Bst is not None:
                    d = st[Bst]
                    den = pstat.tile([1, w2], F32, tag="den")
                    for k in range(NBK):
                        nc.tensor.matmul(
                            den[:], ones_128x1[:], d["ex"][k][:],
                            start=(k == 0), stop=(k == NBK - 1),
                        )
                    d["den"] = den

    nc.compile()
    return nc


# ---------------------------------------------------------------- host side

_RUNNER = None


def _get_runner():
    global _RUNNER
    if _RUNNER is None:
        nc = build_kernel()
        _RUNNER = _BassPjrtRunner(nc, NCORES)
    return _RUNNER


def _split_bf16(x):
    hi = x.astype(ml_dtypes.bfloat16)
    lo = (x - hi.astype(np.float32)).astype(ml_dtypes.bfloat16)
    return hi, lo


def prep_inputs(H, units):
    H = np.ascontiguousarray(np.asarray(H, dtype=np.float32))
    U = np.ascontiguousarray(np.asarray(units, dtype=np.float32))
    h1, h2 = _split_bf16(H)
    u1, u2 = _split_bf16(U)
    msq = (U.astype(np.float64) ** 2).sum(0).astype(np.float32)
    mb = np.stack([-msq, -0.5 * msq], 0).reshape(2, SBK, 128)

    u1 = u1.reshape(DC, 128, S)
    u2 = u2.reshape(DC, 128, S)
    in_maps = []
    for c in range(NCORES):
        sl = slice(c * B_SH, (c + 1) * B_SH)
        in_maps.append({
            "h1": h1[sl].reshape(B_SH, DC, 128, T),
            "h2": h2[sl].reshape(B_SH, DC, 128, T),
            "u1": u1, "u2": u2, "mb": mb,
        })
    return in_maps


def kernel(H, units):
    runner = _get_runner()
    in_maps = prep_inputs(H, units)
    args = runner.prep_inputs(in_maps)
    outs = runner.run(args)
    c = np.asarray(outs[0])           # (NCORES*B_SH, S, T) concat on axis 0
    return c.reshape(B, S, T)


# ------------------------------------------------- embedded PJRT runner

class _BassPjrtRunner:
    def __init__(self, nc, n_cores):
        import jax
        from jax.sharding import Mesh, PartitionSpec
        from jax.experimental.shard_map import shard_map
        from concourse import bass2jax

        bass2jax.install_neuronx_cc_hook()
        self.nc = nc
        self.n_cores = n_cores
        partition_name = (
            nc.partition_id_tensor.name if nc.partition_id_tensor else None
        )
        in_names, out_names, out_avals, zero_outs = [], [], [], []
        for alloc in nc.m.functions[0].allocations:
            if not isinstance(alloc, mybir.MemoryLocationSet):
                continue
            name = alloc.memorylocations[0].name
            if alloc.kind == "ExternalInput":
                if name != partition_name:
                    in_names.append(name)
            elif alloc.kind == "ExternalOutput":
                shape = tuple(alloc.tensor_shape)
                dtype = mybir.dt.np(alloc.dtype)
                out_names.append(name)
                out_avals.append(jax.core.ShapedArray(shape, dtype))
                zero_outs.append((shape, dtype))
        self.in_names = in_names
        self.out_names = out_names
        self.out_shapes = zero_outs
        n_params = len(in_names)
        n_outs = len(out_avals)
        all_in_names = in_names + out_names
        if partition_name is not None:
            all_in_names.append(partition_name)
        self.n_params = n_params

        def _body(*args):
            operands = list(args)
            if partition_name is not None:
                operands.append(bass2jax.partition_id_tensor())
            outs = bass2jax._bass_exec_p.bind(
                *operands,
                out_avals=tuple(out_avals),
                in_names=tuple(all_in_names),
                out_names=tuple(out_names),
                lowering_input_output_aliases=(),
                sim_require_finite=False,
                sim_require_nnan=False,
                nc=nc,
            )
            return tuple(outs)

        devices = jax.devices()[:n_cores]
        assert len(devices) == n_cores
        if n_cores == 1:
            self._fn = jax.jit(_body, keep_unused=True)
        else:
            mesh = Mesh(np.asarray(devices), ("core",))
            in_specs = (PartitionSpec("core"),) * (n_params + n_outs)
            out_specs = (PartitionSpec("core"),) * n_outs
            self._fn = jax.jit(
                shard_map(_body, mesh=mesh, in_specs=in_specs,
                          out_specs=out_specs, check_rep=False),
                keep_unused=True,
            )

    def prep_inputs(self, in_maps):
        per_core = [[np.asarray(m[n]) for n in self.in_names] for m in in_maps]
        if self.n_cores == 1:
            args = per_core[0]
        else:
            args = [
                np.concatenate([per_core[c][i] for c in range(self.n_cores)], 0)
                for i in range(self.n_params)
            ]
        zouts = []
        for (s, d) in self.out_shapes:
            full = (s[0] * self.n_cores,) + tuple(s[1:]) \
                if self.n_cores > 1 else s
            zouts.append(np.zeros(full, d))
        return args + zouts

    def run(self, args):
        import jax
        outs = self._fn(*args)
        jax.block_until_ready(outs)
        return outs


# revision 18
# speedup vs baseline: 14237.7074x; 1.0011x over previous
"""Trainium2 Bass kernel for nn_MemoryBank (vq_codebook softmax).

C[b, s, t] = softmax_s(-||H[b,:,t] - units[:,s]||^2)
           = softmax_s(2*cross[s,t] - m_sq[s]),  cross = units.T @ H[b]

Strategy (8 NeuronCores, data-parallel over batch B=64 -> 8 per core):
  - bf16 3-term split GEMM (h1u1 + h1u2 + h2u1) for ~fp32-accurate logits
    (fp32r 1-pass was measured at 2.4e-2 softmax max-rel -- fails the gate).
  - Layout: s on partitions (output-native), t blocks of 256 on free axis.
    Cross accumulates into PSUM, two s-blocks packed per 512-col bank.
  - Per t-block softmax over s:
      max:  DVE scalar_tensor_tensor chain ((cr - msq/2) max acc) over the
            8 half-banks + GPSIMD partition_all_reduce -> one K=1 bf16
            rank-1 matmul per bank adds -max/2 into PSUM (cancels exactly;
            bf16 rounding of the shift is harmless).
      exp1: ACT Exp(scale=2, bias=-msq fp32 per-partition AP) -> bf16 SBUF.
            Only ACT function used -> no ACT table reloads.
      den:  PE ones-matmul over exp1 -> [1, 512] PSUM; fold halves; DVE
            reciprocal; PE rank-1 broadcasts the [1,256] recip row into a
            PSUM bank.
      out:  DVE multiply exp1 * recip_bcast -> fp32 SBUF -> DMA.
  - The emission is software-pipelined 3 deep (cross(j) | max/exp1/den(j-1)
    | recip/out(j-2)) so the PE FIFO never waits on the GPSIMD/DVE chain;
    PE gaps stay under the ~3.4us HAM window and the PE runs warm (2.4 GHz)
    throughout (unpipelined: 94 HAM throttle episodes, 1.36 ms; pipelined
    + held-back filler MMs: 0.877 ms, PE ~99% busy in-stream).
"""
import numpy as np
import ml_dtypes

import concourse.bacc as bacc
import concourse.bass as bass
import concourse.bass_isa as bass_isa
import concourse.mybir as mybir
import concourse.tile as tile
from concourse.tile import add_dep_helper

F32 = mybir.dt.float32
BF16 = mybir.dt.bfloat16
FP16 = mybir.dt.float16
AF = mybir.ActivationFunctionType
ALU = mybir.AluOpType

# Problem shape (hardcoded per harness contract)
B, D, T, S = 64, 512, 2048, 1024
NCORES = 8
B_SH = B // NCORES          # batches per core
DC = D // 128               # d chunks of 128
SBK = S // 128              # s blocks of 128 (partition dim of output)
TBL = 256                   # t block (free axis) per region
NBK = 4                     # cross banks per group (2 s-blocks each)


def build_kernel(b_sh=B_SH, t=T, tbl=TBL):
    ntb = t // tbl
    nc = bacc.Bacc(None, target_bir_lowering=False, debug=False)

    h1_d = nc.dram_tensor("h1", [b_sh, DC, 128, t], BF16, kind="ExternalInput")
    h2_d = nc.dram_tensor("h2", [b_sh, DC, 128, t], BF16, kind="ExternalInput")
    u1_d = nc.dram_tensor("u1", [DC, 128, S], BF16, kind="ExternalInput")
    u2_d = nc.dram_tensor("u2", [DC, 128, S], BF16, kind="ExternalInput")
    mb_d = nc.dram_tensor("mb", [2, SBK, 128], F32, kind="ExternalInput")
    c_d = nc.dram_tensor("C", [b_sh, S, t], F32, kind="ExternalOutput")

    w2 = 2 * tbl
    steps = [(b, it) for b in range(b_sh) for it in range(ntb)]
    n = len(steps)

    with tile.TileContext(nc) as tc:
        with (
            tc.tile_pool(name="const", bufs=1) as cpool,
            tc.tile_pool(name="hbuf", bufs=2) as hpool,
            tc.tile_pool(name="work", bufs=3) as wpool,
            tc.tile_pool(name="expp", bufs=3) as epool,
            tc.tile_pool(name="outp", bufs=3) as opool,
            tc.tile_pool(name="ps", bufs=2, space="PSUM") as ps,
            tc.tile_pool(name="pstat", bufs=1, space="PSUM") as pstat,
        ):
            u1_sb = cpool.tile([128, DC, S], BF16, tag="u1")
            u2_sb = cpool.tile([128, DC, S], BF16, tag="u2")
            # chunked along s so cross(0) banks 0,1 only wait on the first
            # half of the codebook
            u1v = u1_d.rearrange("c p s -> p c s")
            u2v = u2_d.rearrange("c p s -> p c s")
            for sh in range(2):
                s0, s1 = sh * (S // 2), (sh + 1) * (S // 2)
                nc.sync.dma_start(u1_sb[:, :, s0:s1], u1v[:, :, s0:s1])
                nc.sync.dma_start(u2_sb[:, :, s0:s1], u2v[:, :, s0:s1])
            mb_sb = cpool.tile([128, 2, SBK], F32, tag="mb")
            nc.sync.dma_start(mb_sb[:], mb_d.rearrange("r k p -> p r k"))
            ones_1x128 = cpool.tile([1, 128], BF16, tag="ones_1x128")
            nc.vector.memset(ones_1x128[:], 1.0)
            ones_1x128f = cpool.tile([1, 128], FP16, tag="ones_1x128f")
            nc.vector.memset(ones_1x128f[:], 1.0)
            ones_128x1 = cpool.tile([128, 1], BF16, tag="ones_128x1")
            nc.vector.memset(ones_128x1[:], 1.0)

            hbufs = {}

            def load_h(b):
                # chunked along t so the first cross of a batch only waits
                # on its own t-slice, not the full 4MB batch load
                h1_sb = hpool.tile([128, DC, t], BF16, tag="h1")
                h2_sb = hpool.tile([128, DC, t], BF16, tag="h2")
                h1v = h1_d[b].rearrange("c p t -> p c t")
                h2v = h2_d[b].rearrange("c p t -> p c t")
                for it in range(ntb):
                    t0 = it * tbl
                    nc.sync.dma_start(h1_sb[:, :, t0:t0 + tbl],
                                      h1v[:, :, t0:t0 + tbl])
                    nc.sync.dma_start(h2_sb[:, :, t0:t0 + tbl],
                                      h2v[:, :, t0:t0 + tbl])
                hbufs[b] = (h1_sb, h2_sb)

            load_h(0)
            st = [None] * n    # per-step pipeline state

            def cross_banks(j, banks, holdback=False):
                # holdback: skip the last 2 chunks of the last bank's second
                # half; they are emitted later (PE filler between the rank-1
                # block and cross banks 2,3, giving ACT exp1 slack to finish
                # before the single-buffered banks are rewritten)
                b, it = steps[j]
                t0 = it * tbl
                h1_sb, h2_sb = hbufs[b]
                d = st[j]
                for k in banks:
                    for half in range(2):
                        sb = 2 * k + half
                        s0 = sb * 128
                        reg = d["cr"][k][:, half * tbl:(half + 1) * tbl]
                        for c in range(DC):
                            if (holdback and k == banks[-1] and half == 1
                                    and c >= DC - 2):
                                continue
                            for i, (uu, hh) in enumerate(
                                ((u1_sb, h1_sb), (u1_sb, h2_sb),
                                 (u2_sb, h1_sb))
                            ):
                                nc.tensor.matmul(
                                    reg,
                                    uu[:, c, s0:s0 + 128],
                                    hh[:, c, t0:t0 + tbl],
                                    start=(half == 0 and c == 0 and i == 0),
                                    stop=False,
                                    skip_group_check=True,
                                )

            def cross_rest(j, k):
                # the held-back MMs of bank k's second half
                b, it = steps[j]
                t0 = it * tbl
                h1_sb, h2_sb = hbufs[b]
                d = st[j]
                sb = 2 * k + 1
                s0 = sb * 128
                reg = d["cr"][k][:, tbl:2 * tbl]
                for c in range(DC - 2, DC):
                    for (uu, hh) in ((u1_sb, h1_sb), (u1_sb, h2_sb),
                                     (u2_sb, h1_sb)):
                        nc.tensor.matmul(
                            reg, uu[:, c, s0:s0 + 128], hh[:, c, t0:t0 + tbl],
                            start=False, stop=False, skip_group_check=True,
                        )

            for j in range(n + 2):
                A = j if j < n else None            # cross stage
                Bst = j - 1 if 1 <= j <= n else None    # max/exp1/den stage
                Cst = j - 2 if 2 <= j <= n + 1 else None  # rc_bc/out stage

                # --- 1. max chain (DVE) + partition reduce (GPSIMD), step B ---
                if Bst is not None:
                    d = st[Bst]
                    acc = wpool.tile([128, tbl], F32, tag="acc")
                    first = True
                    for k in range(NBK):
                        for half in range(2):
                            sb = 2 * k + half
                            src = d["cr"][k][:, half * tbl:(half + 1) * tbl]
                            msq2 = mb_sb[:, 1, sb:sb + 1]
                            if first:
                                nc.vector.tensor_scalar(
                                    acc[:], src, msq2, None, op0=ALU.add)
                                first = False
                            else:
                                nc.vector.scalar_tensor_tensor(
                                    acc[:], src, msq2, acc[:],
                                    op0=ALU.add, op1=ALU.max)
                    mbc = wpool.tile([128, tbl], F32, tag="mbc")
                    nc.gpsimd.partition_all_reduce(
                        mbc[:], acc[:], channels=128,
                        reduce_op=bass_isa.ReduceOp.max,
                    )
                    d["mbc"] = mbc

                # --- 2. cross banks 0,1 for step A (+ h prefetch) ---
                if A is not None:
                    b, it = steps[A]
                    st[A] = {"cr": [ps.tile([128, w2], F32, tag=f"cr{k}",
                                            name=f"cr{k}",
                                            bufs=(2 if k < 2 else 1))
                                    for k in range(NBK)]}
                    cross_banks(A, (0, 1), holdback=True)
                    if it == max(ntb - 3, 0) and b + 1 < b_sh:
                        load_h(b + 1)

                # --- 3. den fold + reciprocal row, step C ---
                if Cst is not None:
                    d = st[Cst]
                    dcp = wpool.tile([1, w2], F32, tag="dcp")
                    nc.vector.tensor_copy(dcp[:], d["den"][:])
                    dsum = wpool.tile([1, tbl], F32, tag="dsum")
                    nc.vector.tensor_add(
                        dsum[:], dcp[0:1, 0:tbl], dcp[0:1, tbl:2 * tbl])
                    rcpf = wpool.tile([1, tbl], F32, tag="rcpf")
                    nc.vector.reciprocal(rcpf[:], dsum[:])
                    rc_row = wpool.tile([1, w2], FP16, tag="rc_row")
                    nc.vector.tensor_copy(rc_row[0:1, 0:tbl], rcpf[:])
                    nc.vector.tensor_copy(rc_row[0:1, tbl:w2], rcpf[:])
                    d["rc_row"] = rc_row

                # --- 4. mh row (-max/2, both halves), step B ---
                if Bst is not None:
                    d = st[Bst]
                    mh = wpool.tile([1, w2], BF16, tag="mh")
                    nc.vector.tensor_scalar_mul(
                        mh[0:1, 0:tbl], d["mbc"][0:1, :], -1.0)
                    nc.vector.tensor_scalar_mul(
                        mh[0:1, tbl:w2], d["mbc"][0:1, :], -1.0)
                    d["mh"] = mh

                # --- 5. rank-1 -max/2 + exp1 (bank order 2,3,0,1), step B ---
                if Bst is not None:
                    d = st[Bst]
                    d["ex"] = [None] * NBK
                    for k in (2, 3, 0, 1):
                        r1 = nc.tensor.matmul(
                            d["cr"][k][:], ones_1x128[:], d["mh"][:],
                            start=False, stop=True, skip_group_check=True,
                        )
                        ex = epool.tile([128, w2], BF16, tag=f"ex{k}",
                                        name=f"ex{k}")
                        for half in range(2):
                            sb = 2 * k + half
                            e1 = nc.scalar.activation(
                                ex[:, half * tbl:(half + 1) * tbl],
                                d["cr"][k][:, half * tbl:(half + 1) * tbl],
                                AF.Exp, bias=mb_sb[:, 0, sb:sb + 1], scale=2.0)
                            add_dep_helper(e1.ins, r1.ins, sync=True,
                                           reason="exp1 after rank1 shift")
                        d["ex"][k] = ex

                # --- 5b. held-back bank-1 cross MMs (PE filler), step A ---
                if A is not None:
                    cross_rest(A, 1)

                # --- 6. broadcast recip across partitions (PE), step C ---
                if Cst is not None:
                    d = st[Cst]
                    rc_bc = pstat.tile([128, w2], F32, tag="bc")
                    nc.tensor.matmul(rc_bc[:], ones_1x128f[:],
                                     d["rc_row"][:], start=True, stop=True)
                    d["rc_bc"] = rc_bc

                # --- 7. cross banks 2,3 for step A ---
                if A is not None:
                    cross_banks(A, (2, 3))

                # --- 8. out = exp1 * recip (DVE) + DMA, step C ---
                if Cst is not None:
                    d = st[Cst]
                    b, it = steps[Cst]
                    t0 = it * tbl
                    for k in range(NBK):
                        ot = opool.tile([128, w2], F32, tag=f"ot{k}",
                                        name=f"ot{k}")
                        nc.vector.tensor_mul(ot[:], d["ex"][k][:],
                                             d["rc_bc"][:])
                        for half in range(2):
                            sb = 2 * k + half
                            nc.sync.dma_start(
                                c_d[b, sb * 128:(sb + 1) * 128, t0:t0 + tbl],
                                ot[:, half * tbl:(half + 1) * tbl],
                            )
                    st[Cst] = None

                # --- 9. denominator (PE ones-matmul), step B ---
                if Bst is not None:
                    d = st[Bst]
                    den = pstat.tile([1, w2], F32, tag="den")
                    for k in range(NBK):
                        nc.tensor.matmul(
                            den[:], ones_128x1[:], d["ex"][k][:],
                            start=(k == 0), stop=(k == NBK - 1),
                        )
                    d["den"] = den

    nc.compile()
    return nc


# ---------------------------------------------------------------- host side

_RUNNER = None


def _get_runner():
    global _RUNNER
    if _RUNNER is None:
        nc = build_kernel()
        _RUNNER = _BassPjrtRunner(nc, NCORES)
    return _RUNNER


def _split_bf16(x):
    hi = x.astype(ml_dtypes.bfloat16)
    lo = (x - hi.astype(np.float32)).astype(ml_dtypes.bfloat16)
    return hi, lo


def prep_inputs(H, units):
    H = np.ascontiguousarray(np.asarray(H, dtype=np.float32))
    U = np.ascontiguousarray(np.asarray(units, dtype=np.float32))
    h1, h2 = _split_bf16(H)
    u1, u2 = _split_bf16(U)
    msq = (U.astype(np.float64) ** 2).sum(0).astype(np.float32)
    mb = np.stack([-msq, -0.5 * msq], 0).reshape(2, SBK, 128)

    u1 = u1.reshape(DC, 128, S)
    u2 = u2.reshape(DC, 128, S)
    in_maps = []
    for c in range(NCORES):
        sl = slice(c * B_SH, (c + 1) * B_SH)
        in_maps.append({
            "h1": h1[sl].reshape(B_SH, DC, 128, T),
            "h2": h2[sl].reshape(B_SH, DC, 128, T),
            "u1": u1, "u2": u2, "mb": mb,
        })
    return in_maps


def kernel(H, units):
    runner = _get_runner()
    in_maps = prep_inputs(H, units)
    args = runner.prep_inputs(in_maps)
    outs = runner.run(args)
    c = np.asarray(outs[0])           # (NCORES*B_SH, S, T) concat on axis 0
    return c.reshape(B, S, T)


# ------------------------------------------------- embedded PJRT runner

class _BassPjrtRunner:
    def __init__(self, nc, n_cores):
        import jax
        from jax.sharding import Mesh, PartitionSpec
        from jax.experimental.shard_map import shard_map
        from concourse import bass2jax

        bass2jax.install_neuronx_cc_hook()
        self.nc = nc
        self.n_cores = n_cores
        partition_name = (
            nc.partition_id_tensor.name if nc.partition_id_tensor else None
        )
        in_names, out_names, out_avals, zero_outs = [], [], [], []
        for alloc in nc.m.functions[0].allocations:
            if not isinstance(alloc, mybir.MemoryLocationSet):
                continue
            name = alloc.memorylocations[0].name
            if alloc.kind == "ExternalInput":
                if name != partition_name:
                    in_names.append(name)
            elif alloc.kind == "ExternalOutput":
                shape = tuple(alloc.tensor_shape)
                dtype = mybir.dt.np(alloc.dtype)
                out_names.append(name)
                out_avals.append(jax.core.ShapedArray(shape, dtype))
                zero_outs.append((shape, dtype))
        self.in_names = in_names
        self.out_names = out_names
        self.out_shapes = zero_outs
        n_params = len(in_names)
        n_outs = len(out_avals)
        all_in_names = in_names + out_names
        if partition_name is not None:
            all_in_names.append(partition_name)
        self.n_params = n_params

        def _body(*args):
            operands = list(args)
            if partition_name is not None:
                operands.append(bass2jax.partition_id_tensor())
            outs = bass2jax._bass_exec_p.bind(
                *operands,
                out_avals=tuple(out_avals),
                in_names=tuple(all_in_names),
                out_names=tuple(out_names),
                lowering_input_output_aliases=(),
                sim_require_finite=False,
                sim_require_nnan=False,
                nc=nc,
            )
            return tuple(outs)

        devices = jax.devices()[:n_cores]
        assert len(devices) == n_cores
        if n_cores == 1:
            self._fn = jax.jit(_body, keep_unused=True)
        else:
            mesh = Mesh(np.asarray(devices), ("core",))
            in_specs = (PartitionSpec("core"),) * (n_params + n_outs)
            out_specs = (PartitionSpec("core"),) * n_outs
            self._fn = jax.jit(
                shard_map(_body, mesh=mesh, in_specs=in_specs,
                          out_specs=out_specs, check_rep=False),
                keep_unused=True,
            )

    def prep_inputs(self, in_maps):
        per_core = [[np.asarray(m[n]) for n in self.in_names] for m in in_maps]
        if self.n_cores == 1:
            args = per_core[0]
        else:
            args = [
                np.concatenate([per_core[c][i] for c in range(self.n_cores)], 0)
                for i in range(self.n_params)
            ]
        zouts = []
        for (s, d) in self.out_shapes:
            full = (s[0] * self.n_cores,) + tuple(s[1:]) \
                if self.n_cores > 1 else s
            zouts.append(np.zeros(full, d))
        return args + zouts

    def run(self, args):
        import jax
        outs = self._fn(*args)
        jax.block_until_ready(outs)
        return outs


# revision 22
# speedup vs baseline: 14265.0843x; 1.0019x over previous
"""Trainium2 Bass kernel for nn_MemoryBank (vq_codebook softmax).

C[b, s, t] = softmax_s(-||H[b,:,t] - units[:,s]||^2)
           = softmax_s(2*cross[s,t] - m_sq[s]),  cross = units.T @ H[b]

Strategy (8 NeuronCores, data-parallel over batch B=64 -> 8 per core):
  - bf16 3-term split GEMM (h1u1 + h1u2 + h2u1) for ~fp32-accurate logits
    (fp32r 1-pass was measured at 2.4e-2 softmax max-rel -- fails the gate).
  - Layout: s on partitions (output-native), t blocks of 256 on free axis.
    Cross accumulates into PSUM, two s-blocks packed per 512-col bank.
  - Per t-block softmax over s:
      max:  DVE scalar_tensor_tensor chain ((cr - msq/2) max acc) over the
            8 half-banks + GPSIMD partition_all_reduce -> one K=1 bf16
            rank-1 matmul per bank adds -max/2 into PSUM (cancels exactly;
            bf16 rounding of the shift is harmless).
      exp1: ACT Exp(scale=2, bias=-msq fp32 per-partition AP) -> bf16 SBUF.
            Only ACT function used -> no ACT table reloads.
      den:  PE ones-matmul over exp1 -> [1, 512] PSUM; fold halves; DVE
            reciprocal; PE rank-1 broadcasts the [1,256] recip row into a
            PSUM bank.
      out:  DVE multiply exp1 * recip_bcast -> fp32 SBUF -> DMA.
  - The emission is software-pipelined 3 deep (cross(j) | max/exp1/den(j-1)
    | recip/out(j-2)) so the PE FIFO never waits on the GPSIMD/DVE chain;
    PE gaps stay under the ~3.4us HAM window and the PE runs warm (2.4 GHz)
    throughout (unpipelined: 94 HAM throttle episodes, 1.36 ms; pipelined
    + held-back filler MMs: 0.876 ms, PE ~99% busy in-stream).
"""
import numpy as np
import ml_dtypes

import concourse.bacc as bacc
import concourse.bass as bass
import concourse.bass_isa as bass_isa
import concourse.mybir as mybir
import concourse.tile as tile
from concourse.tile import add_dep_helper

F32 = mybir.dt.float32
BF16 = mybir.dt.bfloat16
FP16 = mybir.dt.float16
AF = mybir.ActivationFunctionType
ALU = mybir.AluOpType

# Problem shape (hardcoded per harness contract)
B, D, T, S = 64, 512, 2048, 1024
NCORES = 8
B_SH = B // NCORES          # batches per core
DC = D // 128               # d chunks of 128
SBK = S // 128              # s blocks of 128 (partition dim of output)
TBL = 256                   # t block (free axis) per region
NBK = 4                     # cross banks per group (2 s-blocks each)


def build_kernel(b_sh=B_SH, t=T, tbl=TBL):
    ntb = t // tbl
    nc = bacc.Bacc(None, target_bir_lowering=False, debug=False)

    h1_d = nc.dram_tensor("h1", [b_sh, DC, 128, t], BF16, kind="ExternalInput")
    h2_d = nc.dram_tensor("h2", [b_sh, DC, 128, t], BF16, kind="ExternalInput")
    u1_d = nc.dram_tensor("u1", [DC, 128, S], BF16, kind="ExternalInput")
    u2_d = nc.dram_tensor("u2", [DC, 128, S], BF16, kind="ExternalInput")
    mb_d = nc.dram_tensor("mb", [2, SBK, 128], F32, kind="ExternalInput")
    c_d = nc.dram_tensor("C", [b_sh, S, t], F32, kind="ExternalOutput")

    w2 = 2 * tbl
    steps = [(b, it) for b in range(b_sh) for it in range(ntb)]
    n = len(steps)

    with tile.TileContext(nc) as tc:
        with (
            tc.tile_pool(name="const", bufs=1) as cpool,
            tc.tile_pool(name="hbuf", bufs=2) as hpool,
            tc.tile_pool(name="work", bufs=3) as wpool,
            tc.tile_pool(name="expp", bufs=3) as epool,
            tc.tile_pool(name="outp", bufs=3) as opool,
            tc.tile_pool(name="ps", bufs=2, space="PSUM") as ps,
            tc.tile_pool(name="pstat", bufs=1, space="PSUM") as pstat,
        ):
            u1_sb = cpool.tile([128, DC, S], BF16, tag="u1")
            u2_sb = cpool.tile([128, DC, S], BF16, tag="u2")
            # DMA order matters (the queue serializes): everything cross(0)
            # banks 0,1 needs comes first -- codebook first halves + bias +
            # (below) batch-0's first t-chunks -- the second codebook halves
            # are deferred until after load_h(0)
            u1v = u1_d.rearrange("c p s -> p c s")
            u2v = u2_d.rearrange("c p s -> p c s")
            nc.sync.dma_start(u1_sb[:, :, 0:S // 2], u1v[:, :, 0:S // 2])
            nc.sync.dma_start(u2_sb[:, :, 0:S // 2], u2v[:, :, 0:S // 2])
            mb_sb = cpool.tile([128, 2, SBK], F32, tag="mb")
            nc.sync.dma_start(mb_sb[:], mb_d.rearrange("r k p -> p r k"))
            ones_1x128 = cpool.tile([1, 128], BF16, tag="ones_1x128")
            nc.vector.memset(ones_1x128[:], 1.0)
            ones_1x128f = cpool.tile([1, 128], FP16, tag="ones_1x128f")
            nc.vector.memset(ones_1x128f[:], 1.0)
            ones_128x1 = cpool.tile([128, 1], BF16, tag="ones_128x1")
            nc.vector.memset(ones_128x1[:], 1.0)

            hbufs = {}

            def load_h(b, its=None):
                # chunked along t so the first cross of a batch only waits
                # on its own t-slice, not the full 4MB batch load
                if its is None:
                    its = range(ntb)
                    h1_sb = hpool.tile([128, DC, t], BF16, tag="h1")
                    h2_sb = hpool.tile([128, DC, t], BF16, tag="h2")
                    hbufs[b] = (h1_sb, h2_sb)
                else:
                    h1_sb, h2_sb = hbufs[b]
                h1v = h1_d[b].rearrange("c p t -> p c t")
                h2v = h2_d[b].rearrange("c p t -> p c t")
                for it in its:
                    t0 = it * tbl
                    nc.sync.dma_start(h1_sb[:, :, t0:t0 + tbl],
                                      h1v[:, :, t0:t0 + tbl])
                    nc.sync.dma_start(h2_sb[:, :, t0:t0 + tbl],
                                      h2v[:, :, t0:t0 + tbl])

            # batch 0: first t-chunk, then the deferred codebook halves,
            # then the rest of the batch
            h1_sb0 = hpool.tile([128, DC, t], BF16, tag="h1")
            h2_sb0 = hpool.tile([128, DC, t], BF16, tag="h2")
            hbufs[0] = (h1_sb0, h2_sb0)
            load_h(0, its=(0,))
            nc.sync.dma_start(u1_sb[:, :, S // 2:S], u1v[:, :, S // 2:S])
            nc.sync.dma_start(u2_sb[:, :, S // 2:S], u2v[:, :, S // 2:S])
            load_h(0, its=range(1, ntb))
            st = [None] * n    # per-step pipeline state

            def cross_banks(j, banks, holdback=False):
                # holdback: skip the last 2 chunks of the last bank's second
                # half; they are emitted later (PE filler between the rank-1
                # block and cross banks 2,3, giving ACT exp1 slack to finish
                # before the single-buffered banks are rewritten)
                b, it = steps[j]
                t0 = it * tbl
                h1_sb, h2_sb = hbufs[b]
                d = st[j]
                for k in banks:
                    for half in range(2):
                        sb = 2 * k + half
                        s0 = sb * 128
                        reg = d["cr"][k][:, half * tbl:(half + 1) * tbl]
                        for c in range(DC):
                            if (holdback and k == banks[-1] and half == 1
                                    and c >= DC - 2):
                                continue
                            for i, (uu, hh) in enumerate(
                                ((u1_sb, h1_sb), (u1_sb, h2_sb),
                                 (u2_sb, h1_sb))
                            ):
                                nc.tensor.matmul(
                                    reg,
                                    uu[:, c, s0:s0 + 128],
                                    hh[:, c, t0:t0 + tbl],
                                    start=(half == 0 and c == 0 and i == 0),
                                    stop=False,
                                    skip_group_check=True,
                                )

            def cross_rest(j, k):
                # the held-back MMs of bank k's second half
                b, it = steps[j]
                t0 = it * tbl
                h1_sb, h2_sb = hbufs[b]
                d = st[j]
                sb = 2 * k + 1
                s0 = sb * 128
                reg = d["cr"][k][:, tbl:2 * tbl]
                for c in range(DC - 2, DC):
                    for (uu, hh) in ((u1_sb, h1_sb), (u1_sb, h2_sb),
                                     (u2_sb, h1_sb)):
                        nc.tensor.matmul(
                            reg, uu[:, c, s0:s0 + 128], hh[:, c, t0:t0 + tbl],
                            start=False, stop=False, skip_group_check=True,
                        )

            for j in range(n + 2):
                A = j if j < n else None            # cross stage
                Bst = j - 1 if 1 <= j <= n else None    # max/exp1/den stage
                Cst = j - 2 if 2 <= j <= n + 1 else None  # rc_bc/out stage

                # --- 1. max chain (DVE) + partition reduce (GPSIMD), step B ---
                if Bst is not None:
                    d = st[Bst]
                    acc = wpool.tile([128, tbl], F32, tag="acc")
                    first = True
                    for k in range(NBK):
                        for half in range(2):
                            sb = 2 * k + half
                            src = d["cr"][k][:, half * tbl:(half + 1) * tbl]
                            msq2 = mb_sb[:, 1, sb:sb + 1]
                            if first:
                                nc.vector.tensor_scalar(
                                    acc[:], src, msq2, None, op0=ALU.add)
                                first = False
                            else:
                                nc.vector.scalar_tensor_tensor(
                                    acc[:], src, msq2, acc[:],
                                    op0=ALU.add, op1=ALU.max)
                    mbc = wpool.tile([128, tbl], F32, tag="mbc")
                    nc.gpsimd.partition_all_reduce(
                        mbc[:], acc[:], channels=128,
                        reduce_op=bass_isa.ReduceOp.max,
                    )
                    d["mbc"] = mbc

                # --- 2. cross banks 0,1 for step A (+ h prefetch) ---
                if A is not None:
                    b, it = steps[A]
                    st[A] = {"cr": [ps.tile([128, w2], F32, tag=f"cr{k}",
                                            name=f"cr{k}",
                                            bufs=(2 if k < 2 else 1))
                                    for k in range(NBK)]}
                    cross_banks(A, (0, 1), holdback=True)
                    if it == max(ntb - 3, 0) and b + 1 < b_sh:
                        load_h(b + 1)

                # --- 3. den fold + reciprocal row, step C ---
                if Cst is not None:
                    d = st[Cst]
                    dcp = wpool.tile([1, w2], F32, tag="dcp")
                    nc.vector.tensor_copy(dcp[:], d["den"][:])
                    dsum = wpool.tile([1, tbl], F32, tag="dsum")
                    nc.vector.tensor_add(
                        dsum[:], dcp[0:1, 0:tbl], dcp[0:1, tbl:2 * tbl])
                    rcpf = wpool.tile([1, tbl], F32, tag="rcpf")
                    nc.vector.reciprocal(rcpf[:], dsum[:])
                    rc_row = wpool.tile([1, w2], FP16, tag="rc_row")
                    nc.vector.tensor_copy(rc_row[0:1, 0:tbl], rcpf[:])
                    nc.vector.tensor_copy(rc_row[0:1, tbl:w2], rcpf[:])
                    d["rc_row"] = rc_row

                # --- 4. mh row (-max/2, both halves), step B ---
                if Bst is not None:
                    d = st[Bst]
                    mh = wpool.tile([1, w2], BF16, tag="mh")
                    nc.vector.tensor_scalar_mul(
                        mh[0:1, 0:tbl], d["mbc"][0:1, :], -1.0)
                    nc.vector.tensor_scalar_mul(
                        mh[0:1, tbl:w2], d["mbc"][0:1, :], -1.0)
                    d["mh"] = mh

                # --- 5. rank-1 -max/2 + exp1 (bank order 2,3,0,1), step B ---
                if Bst is not None:
                    d = st[Bst]
                    d["ex"] = [None] * NBK
                    for k in (2, 3, 0, 1):
                        r1 = nc.tensor.matmul(
                            d["cr"][k][:], ones_1x128[:], d["mh"][:],
                            start=False, stop=True, skip_group_check=True,
                        )
                        ex = epool.tile([128, w2], BF16, tag=f"ex{k}",
                                        name=f"ex{k}")
                        for half in range(2):
                            sb = 2 * k + half
                            e1 = nc.scalar.activation(
                                ex[:, half * tbl:(half + 1) * tbl],
                                d["cr"][k][:, half * tbl:(half + 1) * tbl],
                                AF.Exp, bias=mb_sb[:, 0, sb:sb + 1], scale=2.0)
                            add_dep_helper(e1.ins, r1.ins, sync=True,
                                           reason="exp1 after rank1 shift")
                        d["ex"][k] = ex

                # --- 5b. held-back bank-1 cross MMs (PE filler), step A ---
                if A is not None:
                    cross_rest(A, 1)

                # --- 6. broadcast recip across partitions (PE), step C ---
                if Cst is not None:
                    d = st[Cst]
                    rc_bc = pstat.tile([128, w2], F32, tag="bc")
                    nc.tensor.matmul(rc_bc[:], ones_1x128f[:],
                                     d["rc_row"][:], start=True, stop=True)
                    d["rc_bc"] = rc_bc

                # --- 7. cross banks 2,3 for step A ---
                if A is not None:
                    cross_banks(A, (2, 3))

                # --- 8. out = exp1 * recip (DVE) + DMA, step C ---
                if Cst is not None:
                    d = st[Cst]
                    b, it = steps[Cst]
                    t0 = it * tbl
                    for k in range(NBK):
                        ot = opool.tile([128, w2], F32, tag=f"ot{k}",
                                        name=f"ot{k}")
                        nc.vector.tensor_mul(ot[:], d["ex"][k][:],
                                             d["rc_bc"][:])
                        for half in range(2):
                            sb = 2 * k + half
                            nc.sync.dma_start(
                                c_d[b, sb * 128:(sb + 1) * 128, t0:t0 + tbl],
                                ot[:, half * tbl:(half + 1) * tbl],
                            )
                    st[Cst] = None

                # --- 9. denominator (PE ones-matmul), step B ---
                if Bst is not None:
                    d = st[Bst]
                    den = pstat.tile([1, w2], F32, tag="den")
                    for k in range(NBK):
                        nc.tensor.matmul(
                            den[:], ones_128x1[:], d["ex"][k][:],
                            start=(k == 0), stop=(k == NBK - 1),
                        )
                    d["den"] = den

    nc.compile()
    return nc


# ---------------------------------------------------------------- host side

_RUNNER = None


def _get_runner():
    global _RUNNER
    if _RUNNER is None:
        nc = build_kernel()
        _RUNNER = _BassPjrtRunner(nc, NCORES)
    return _RUNNER


def _split_bf16(x):
    hi = x.astype(ml_dtypes.bfloat16)
    lo = (x - hi.astype(np.float32)).astype(ml_dtypes.bfloat16)
    return hi, lo


def prep_inputs(H, units):
    H = np.ascontiguousarray(np.asarray(H, dtype=np.float32))
    U = np.ascontiguousarray(np.asarray(units, dtype=np.float32))
    h1, h2 = _split_bf16(H)
    u1, u2 = _split_bf16(U)
    msq = (U.astype(np.float64) ** 2).sum(0).astype(np.float32)
    mb = np.stack([-msq, -0.5 * msq], 0).reshape(2, SBK, 128)

    u1 = u1.reshape(DC, 128, S)
    u2 = u2.reshape(DC, 128, S)
    in_maps = []
    for c in range(NCORES):
        sl = slice(c * B_SH, (c + 1) * B_SH)
        in_maps.append({
            "h1": h1[sl].reshape(B_SH, DC, 128, T),
            "h2": h2[sl].reshape(B_SH, DC, 128, T),
            "u1": u1, "u2": u2, "mb": mb,
        })
    return in_maps


def kernel(H, units):
    runner = _get_runner()
    in_maps = prep_inputs(H, units)
    args = runner.prep_inputs(in_maps)
    outs = runner.run(args)
    c = np.asarray(outs[0])           # (NCORES*B_SH, S, T) concat on axis 0
    return c.reshape(B, S, T)


# ------------------------------------------------- embedded PJRT runner

class _BassPjrtRunner:
    def __init__(self, nc, n_cores):
        import jax
        from jax.sharding import Mesh, PartitionSpec
        from jax.experimental.shard_map import shard_map
        from concourse import bass2jax

        bass2jax.install_neuronx_cc_hook()
        self.nc = nc
        self.n_cores = n_cores
        partition_name = (
            nc.partition_id_tensor.name if nc.partition_id_tensor else None
        )
        in_names, out_names, out_avals, zero_outs = [], [], [], []
        for alloc in nc.m.functions[0].allocations:
            if not isinstance(alloc, mybir.MemoryLocationSet):
                continue
            name = alloc.memorylocations[0].name
            if alloc.kind == "ExternalInput":
                if name != partition_name:
                    in_names.append(name)
            elif alloc.kind == "ExternalOutput":
                shape = tuple(alloc.tensor_shape)
                dtype = mybir.dt.np(alloc.dtype)
                out_names.append(name)
                out_avals.append(jax.core.ShapedArray(shape, dtype))
                zero_outs.append((shape, dtype))
        self.in_names = in_names
        self.out_names = out_names
        self.out_shapes = zero_outs
        n_params = len(in_names)
        n_outs = len(out_avals)
        all_in_names = in_names + out_names
        if partition_name is not None:
            all_in_names.append(partition_name)
        self.n_params = n_params

        def _body(*args):
            operands = list(args)
            if partition_name is not None:
                operands.append(bass2jax.partition_id_tensor())
            outs = bass2jax._bass_exec_p.bind(
                *operands,
                out_avals=tuple(out_avals),
                in_names=tuple(all_in_names),
                out_names=tuple(out_names),
                lowering_input_output_aliases=(),
                sim_require_finite=False,
                sim_require_nnan=False,
                nc=nc,
            )
            return tuple(outs)

        devices = jax.devices()[:n_cores]
        assert len(devices) == n_cores
        if n_cores == 1:
            self._fn = jax.jit(_body, keep_unused=True)
        else:
            mesh = Mesh(np.asarray(devices), ("core",))
            in_specs = (PartitionSpec("core"),) * (n_params + n_outs)
            out_specs = (PartitionSpec("core"),) * n_outs
            self._fn = jax.jit(
                shard_map(_body, mesh=mesh, in_specs=in_specs,
                          out_specs=out_specs, check_rep=False),
                keep_unused=True,
            )

    def prep_inputs(self, in_maps):
        per_core = [[np.asarray(m[n]) for n in self.in_names] for m in in_maps]
        if self.n_cores == 1:
            args = per_core[0]
        else:
            args = [
                np.concatenate([per_core[c][i] for c in range(self.n_cores)], 0)
                for i in range(self.n_params)
            ]
        zouts = []
        for (s, d) in self.out_shapes:
            full = (s[0] * self.n_cores,) + tuple(s[1:]) \
                if self.n_cores > 1 else s
            zouts.append(np.zeros(full, d))
        return args + zouts

    def run(self, args):
        import jax
        outs = self._fn(*args)
        jax.block_until_ready(outs)
        return outs


# revision 23
# speedup vs baseline: 14280.5608x; 1.0011x over previous
"""Trainium2 Bass kernel for nn_MemoryBank (vq_codebook softmax).

C[b, s, t] = softmax_s(-||H[b,:,t] - units[:,s]||^2)
           = softmax_s(2*cross[s,t] - m_sq[s]),  cross = units.T @ H[b]

Strategy (8 NeuronCores, data-parallel over batch B=64 -> 8 per core):
  - bf16 3-term split GEMM (h1u1 + h1u2 + h2u1) for ~fp32-accurate logits
    (fp32r 1-pass was measured at 2.4e-2 softmax max-rel -- fails the gate).
  - Layout: s on partitions (output-native), t blocks of 256 on free axis.
    Cross accumulates into PSUM, two s-blocks packed per 512-col bank.
  - Per t-block softmax over s:
      max:  DVE scalar_tensor_tensor chain ((cr - msq/2) max acc) over the
            8 half-banks + GPSIMD partition_all_reduce -> one K=1 bf16
            rank-1 matmul per bank adds -max/2 into PSUM (cancels exactly;
            bf16 rounding of the shift is harmless).
      exp1: ACT Exp(scale=2, bias=-msq fp32 per-partition AP) -> bf16 SBUF.
            Only ACT function used -> no ACT table reloads.
      den:  PE ones-matmul over exp1 -> [1, 512] PSUM; fold halves; DVE
            reciprocal; PE rank-1 broadcasts the [1,256] recip row into a
            PSUM bank.
      out:  DVE multiply exp1 * recip_bcast -> fp32 SBUF -> DMA.
  - The emission is software-pipelined 3 deep (cross(j) | max/exp1/den(j-1)
    | recip/out(j-2)) so the PE FIFO never waits on the GPSIMD/DVE chain;
    PE gaps stay under the ~3.4us HAM window and the PE runs warm (2.4 GHz)
    throughout (unpipelined: 94 HAM throttle episodes, 1.36 ms; pipelined
    + held-back filler MMs: 0.876 ms, PE ~99% busy in-stream).
"""
import numpy as np
import ml_dtypes

import concourse.bacc as bacc
import concourse.bass as bass
import concourse.bass_isa as bass_isa
import concourse.mybir as mybir
import concourse.tile as tile
from concourse.tile import add_dep_helper

F32 = mybir.dt.float32
BF16 = mybir.dt.bfloat16
FP16 = mybir.dt.float16
AF = mybir.ActivationFunctionType
ALU = mybir.AluOpType

# Problem shape (hardcoded per harness contract)
B, D, T, S = 64, 512, 2048, 1024
NCORES = 8
B_SH = B // NCORES          # batches per core
DC = D // 128               # d chunks of 128
SBK = S // 128              # s blocks of 128 (partition dim of output)
TBL = 256                   # t block (free axis) per region
NBK = 4                     # cross banks per group (2 s-blocks each)


def build_kernel(b_sh=B_SH, t=T, tbl=TBL):
    ntb = t // tbl
    nc = bacc.Bacc(None, target_bir_lowering=False, debug=False)

    h1_d = nc.dram_tensor("h1", [b_sh, DC, 128, t], BF16, kind="ExternalInput")
    h2_d = nc.dram_tensor("h2", [b_sh, DC, 128, t], BF16, kind="ExternalInput")
    u1_d = nc.dram_tensor("u1", [DC, 128, S], BF16, kind="ExternalInput")
    u2_d = nc.dram_tensor("u2", [DC, 128, S], BF16, kind="ExternalInput")
    mb_d = nc.dram_tensor("mb", [2, SBK, 128], F32, kind="ExternalInput")
    c_d = nc.dram_tensor("C", [b_sh, S, t], F32, kind="ExternalOutput")

    w2 = 2 * tbl
    steps = [(b, it) for b in range(b_sh) for it in range(ntb)]
    n = len(steps)

    with tile.TileContext(nc) as tc:
        with (
            tc.tile_pool(name="const", bufs=1) as cpool,
            tc.tile_pool(name="hbuf", bufs=2) as hpool,
            tc.tile_pool(name="work", bufs=3) as wpool,
            tc.tile_pool(name="expp", bufs=3) as epool,
            tc.tile_pool(name="outp", bufs=3) as opool,
            tc.tile_pool(name="ps", bufs=2, space="PSUM") as ps,
            tc.tile_pool(name="pstat", bufs=1, space="PSUM") as pstat,
        ):
            u1_sb = cpool.tile([128, DC, S], BF16, tag="u1")
            u2_sb = cpool.tile([128, DC, S], BF16, tag="u2")
            # DMA order matters (the queue serializes): everything cross(0)
            # banks 0,1 needs comes first -- codebook first halves + bias +
            # (below) batch-0's first t-chunks -- the second codebook halves
            # are deferred until after load_h(0)
            u1v = u1_d.rearrange("c p s -> p c s")
            u2v = u2_d.rearrange("c p s -> p c s")
            nc.sync.dma_start(u1_sb[:, :, 0:S // 2], u1v[:, :, 0:S // 2])
            nc.sync.dma_start(u2_sb[:, :, 0:S // 2], u2v[:, :, 0:S // 2])
            mb_sb = cpool.tile([128, 2, SBK], F32, tag="mb")
            nc.sync.dma_start(mb_sb[:], mb_d.rearrange("r k p -> p r k"))
            ones_1x128 = cpool.tile([1, 128], BF16, tag="ones_1x128")
            nc.vector.memset(ones_1x128[:], 1.0)
            ones_1x128f = cpool.tile([1, 128], FP16, tag="ones_1x128f")
            nc.vector.memset(ones_1x128f[:], 1.0)
            ones_128x1 = cpool.tile([128, 1], BF16, tag="ones_128x1")
            nc.vector.memset(ones_128x1[:], 1.0)

            hbufs = {}

            def load_h(b, its=None):
                # chunked along t so the first cross of a batch only waits
                # on its own t-slice, not the full 4MB batch load
                if its is None:
                    its = range(ntb)
                    h1_sb = hpool.tile([128, DC, t], BF16, tag="h1")
                    h2_sb = hpool.tile([128, DC, t], BF16, tag="h2")
                    hbufs[b] = (h1_sb, h2_sb)
                else:
                    h1_sb, h2_sb = hbufs[b]
                h1v = h1_d[b].rearrange("c p t -> p c t")
                h2v = h2_d[b].rearrange("c p t -> p c t")
                for it in its:
                    t0 = it * tbl
                    nc.sync.dma_start(h1_sb[:, :, t0:t0 + tbl],
                                      h1v[:, :, t0:t0 + tbl])
                    nc.sync.dma_start(h2_sb[:, :, t0:t0 + tbl],
                                      h2v[:, :, t0:t0 + tbl])

            # batch 0: first t-chunk, then the deferred codebook halves,
            # then the rest of the batch
            h1_sb0 = hpool.tile([128, DC, t], BF16, tag="h1")
            h2_sb0 = hpool.tile([128, DC, t], BF16, tag="h2")
            hbufs[0] = (h1_sb0, h2_sb0)
            load_h(0, its=(0,))
            nc.sync.dma_start(u1_sb[:, :, S // 2:S], u1v[:, :, S // 2:S])
            nc.sync.dma_start(u2_sb[:, :, S // 2:S], u2v[:, :, S // 2:S])
            load_h(0, its=range(1, ntb))
            st = [None] * n    # per-step pipeline state

            def cross_banks(j, banks, holdback=False):
                # holdback: skip the last 2 chunks of the last bank's second
                # half; they are emitted later (PE filler between the rank-1
                # block and cross banks 2,3, giving ACT exp1 slack to finish
                # before the single-buffered banks are rewritten)
                b, it = steps[j]
                t0 = it * tbl
                h1_sb, h2_sb = hbufs[b]
                d = st[j]
                for k in banks:
                    for half in range(2):
                        sb = 2 * k + half
                        s0 = sb * 128
                        reg = d["cr"][k][:, half * tbl:(half + 1) * tbl]
                        for c in range(DC):
                            if (holdback and k == banks[-1] and half == 1
                                    and c >= DC - 2):
                                continue
                            for i, (uu, hh) in enumerate(
                                ((u1_sb, h1_sb), (u1_sb, h2_sb),
                                 (u2_sb, h1_sb))
                            ):
                                nc.tensor.matmul(
                                    reg,
                                    uu[:, c, s0:s0 + 128],
                                    hh[:, c, t0:t0 + tbl],
                                    start=(half == 0 and c == 0 and i == 0),
                                    stop=False,
                                    skip_group_check=True,
                                )

            def cross_rest(j, k):
                # the held-back MMs of bank k's second half
                b, it = steps[j]
                t0 = it * tbl
                h1_sb, h2_sb = hbufs[b]
                d = st[j]
                sb = 2 * k + 1
                s0 = sb * 128
                reg = d["cr"][k][:, tbl:2 * tbl]
                for c in range(DC - 2, DC):
                    for (uu, hh) in ((u1_sb, h1_sb), (u1_sb, h2_sb),
                                     (u2_sb, h1_sb)):
                        nc.tensor.matmul(
                            reg, uu[:, c, s0:s0 + 128], hh[:, c, t0:t0 + tbl],
                            start=False, stop=False, skip_group_check=True,
                        )

            for j in range(n + 2):
                A = j if j < n else None            # cross stage
                Bst = j - 1 if 1 <= j <= n else None    # max/exp1/den stage
                Cst = j - 2 if 2 <= j <= n + 1 else None  # rc_bc/out stage

                # --- 1. max chain (DVE) + partition reduce (GPSIMD), step B ---
                if Bst is not None and "mbc" not in st[Bst]:
                    d = st[Bst]
                    acc = wpool.tile([128, tbl], F32, tag="acc")
                    first = True
                    for k in range(NBK):
                        for half in range(2):
                            sb = 2 * k + half
                            src = d["cr"][k][:, half * tbl:(half + 1) * tbl]
                            msq2 = mb_sb[:, 1, sb:sb + 1]
                            if first:
                                nc.vector.tensor_scalar(
                                    acc[:], src, msq2, None, op0=ALU.add)
                                first = False
                            else:
                                nc.vector.scalar_tensor_tensor(
                                    acc[:], src, msq2, acc[:],
                                    op0=ALU.add, op1=ALU.max)
                    mbc = wpool.tile([128, tbl], F32, tag="mbc")
                    nc.gpsimd.partition_all_reduce(
                        mbc[:], acc[:], channels=128,
                        reduce_op=bass_isa.ReduceOp.max,
                    )
                    d["mbc"] = mbc

                # --- 2. cross banks 0,1 for step A (+ h prefetch) ---
                if A is not None:
                    b, it = steps[A]
                    st[A] = {"cr": [ps.tile([128, w2], F32, tag=f"cr{k}",
                                            name=f"cr{k}",
                                            bufs=(2 if k < 2 else 1))
                                    for k in range(NBK)]}
                    cross_banks(A, (0, 1), holdback=True)
                    if it == max(ntb - 3, 0) and b + 1 < b_sh:
                        load_h(b + 1)

                # --- 3. den fold + reciprocal row, step C ---
                if Cst is not None:
                    d = st[Cst]
                    dcp = wpool.tile([1, w2], F32, tag="dcp")
                    nc.vector.tensor_copy(dcp[:], d["den"][:])
                    dsum = wpool.tile([1, tbl], F32, tag="dsum")
                    nc.vector.tensor_add(
                        dsum[:], dcp[0:1, 0:tbl], dcp[0:1, tbl:2 * tbl])
                    rcpf = wpool.tile([1, tbl], F32, tag="rcpf")
                    nc.vector.reciprocal(rcpf[:], dsum[:])
                    rc_row = wpool.tile([1, w2], FP16, tag="rc_row")
                    nc.vector.tensor_copy(rc_row[0:1, 0:tbl], rcpf[:])
                    nc.vector.tensor_copy(rc_row[0:1, tbl:w2], rcpf[:])
                    d["rc_row"] = rc_row

                # --- 4. mh row (-max/2, both halves), step B ---
                if Bst is not None:
                    d = st[Bst]
                    mh = wpool.tile([1, w2], BF16, tag="mh")
                    nc.vector.tensor_scalar_mul(
                        mh[0:1, 0:tbl], d["mbc"][0:1, :], -1.0)
                    nc.vector.tensor_scalar_mul(
                        mh[0:1, tbl:w2], d["mbc"][0:1, :], -1.0)
                    d["mh"] = mh

                # --- 5. rank-1 -max/2 + exp1 (bank order 2,3,0,1), step B ---
                if Bst is not None:
                    d = st[Bst]
                    d["ex"] = [None] * NBK
                    for k in (2, 3, 0, 1):
                        r1 = nc.tensor.matmul(
                            d["cr"][k][:], ones_1x128[:], d["mh"][:],
                            start=False, stop=True, skip_group_check=True,
                        )
                        ex = epool.tile([128, w2], BF16, tag=f"ex{k}",
                                        name=f"ex{k}")
                        for half in range(2):
                            sb = 2 * k + half
                            e1 = nc.scalar.activation(
                                ex[:, half * tbl:(half + 1) * tbl],
                                d["cr"][k][:, half * tbl:(half + 1) * tbl],
                                AF.Exp, bias=mb_sb[:, 0, sb:sb + 1], scale=2.0)
                            add_dep_helper(e1.ins, r1.ins, sync=True,
                                           reason="exp1 after rank1 shift")
                        d["ex"][k] = ex

                # --- 5b. held-back bank-1 cross MMs (PE filler), step A ---
                if A is not None:
                    cross_rest(A, 1)

                # --- 6. broadcast recip across partitions (PE), step C ---
                if Cst is not None:
                    d = st[Cst]
                    rc_bc = pstat.tile([128, w2], F32, tag="bc")
                    nc.tensor.matmul(rc_bc[:], ones_1x128f[:],
                                     d["rc_row"][:], start=True, stop=True)
                    d["rc_bc"] = rc_bc

                # --- 7. cross banks 2,3 for step A ---
                if A is not None:
                    cross_banks(A, (2, 3))
                    if A == n - 1:
                        # final step: emit its max chain now so the pipeline
                        # drain does not wait on GPSIMD with an idle PE
                        d = st[A]
                        acc = wpool.tile([128, tbl], F32, tag="acc")
                        first = True
                        for k in range(NBK):
                            for half in range(2):
                                sb = 2 * k + half
                                src_ap = d["cr"][k][:,
                                                    half * tbl:(half + 1) * tbl]
                                msq2 = mb_sb[:, 1, sb:sb + 1]
                                if first:
                                    nc.vector.tensor_scalar(
                                        acc[:], src_ap, msq2, None,
                                        op0=ALU.add)
                                    first = False
                                else:
                                    nc.vector.scalar_tensor_tensor(
                                        acc[:], src_ap, msq2, acc[:],
                                        op0=ALU.add, op1=ALU.max)
                        mbc = wpool.tile([128, tbl], F32, tag="mbc")
                        nc.gpsimd.partition_all_reduce(
                            mbc[:], acc[:], channels=128,
                            reduce_op=bass_isa.ReduceOp.max,
                        )
                        d["mbc"] = mbc

                # --- 8. out = exp1 * recip (DVE) + DMA, step C ---
                if Cst is not None:
                    d = st[Cst]
                    b, it = steps[Cst]
                    t0 = it * tbl
                    for k in range(NBK):
                        ot = opool.tile([128, w2], F32, tag=f"ot{k}",
                                        name=f"ot{k}")
                        nc.vector.tensor_mul(ot[:], d["ex"][k][:],
                                             d["rc_bc"][:])
                        for half in range(2):
                            sb = 2 * k + half
                            nc.sync.dma_start(
                                c_d[b, sb * 128:(sb + 1) * 128, t0:t0 + tbl],
                                ot[:, half * tbl:(half + 1) * tbl],
                            )
                    st[Cst] = None

                # --- 9. denominator (PE ones-matmul), step B ---
                if Bst is not None:
                    d = st[Bst]
                    den = pstat.tile([1, w2], F32, tag="den")
                    for k in range(NBK):
                        nc.tensor.matmul(
                            den[:], ones_128x1[:], d["ex"][k][:],
                            start=(k == 0), stop=(k == NBK - 1),
                        )
                    d["den"] = den

    nc.compile()
    return nc


# ---------------------------------------------------------------- host side

_RUNNER = None


def _get_runner():
    global _RUNNER
    if _RUNNER is None:
        nc = build_kernel()
        _RUNNER = _BassPjrtRunner(nc, NCORES)
    return _RUNNER


def _split_bf16(x):
    hi = x.astype(ml_dtypes.bfloat16)
    lo = (x - hi.astype(np.float32)).astype(ml_dtypes.bfloat16)
    return hi, lo


def prep_inputs(H, units):
    H = np.ascontiguousarray(np.asarray(H, dtype=np.float32))
    U = np.ascontiguousarray(np.asarray(units, dtype=np.float32))
    h1, h2 = _split_bf16(H)
    u1, u2 = _split_bf16(U)
    msq = (U.astype(np.float64) ** 2).sum(0).astype(np.float32)
    mb = np.stack([-msq, -0.5 * msq], 0).reshape(2, SBK, 128)

    u1 = u1.reshape(DC, 128, S)
    u2 = u2.reshape(DC, 128, S)
    in_maps = []
    for c in range(NCORES):
        sl = slice(c * B_SH, (c + 1) * B_SH)
        in_maps.append({
            "h1": h1[sl].reshape(B_SH, DC, 128, T),
            "h2": h2[sl].reshape(B_SH, DC, 128, T),
            "u1": u1, "u2": u2, "mb": mb,
        })
    return in_maps


def kernel(H, units):
    runner = _get_runner()
    in_maps = prep_inputs(H, units)
    args = runner.prep_inputs(in_maps)
    outs = runner.run(args)
    c = np.asarray(outs[0])           # (NCORES*B_SH, S, T) concat on axis 0
    return c.reshape(B, S, T)


# ------------------------------------------------- embedded PJRT runner

class _BassPjrtRunner:
    def __init__(self, nc, n_cores):
        import jax
        from jax.sharding import Mesh, PartitionSpec
        from jax.experimental.shard_map import shard_map
        from concourse import bass2jax

        bass2jax.install_neuronx_cc_hook()
        self.nc = nc
        self.n_cores = n_cores
        partition_name = (
            nc.partition_id_tensor.name if nc.partition_id_tensor else None
        )
        in_names, out_names, out_avals, zero_outs = [], [], [], []
        for alloc in nc.m.functions[0].allocations:
            if not isinstance(alloc, mybir.MemoryLocationSet):
                continue
            name = alloc.memorylocations[0].name
            if alloc.kind == "ExternalInput":
                if name != partition_name:
                    in_names.append(name)
            elif alloc.kind == "ExternalOutput":
                shape = tuple(alloc.tensor_shape)
                dtype = mybir.dt.np(alloc.dtype)
                out_names.append(name)
                out_avals.append(jax.core.ShapedArray(shape, dtype))
                zero_outs.append((shape, dtype))
        self.in_names = in_names
        self.out_names = out_names
        self.out_shapes = zero_outs
        n_params = len(in_names)
        n_outs = len(out_avals)
        all_in_names = in_names + out_names
        if partition_name is not None:
            all_in_names.append(partition_name)
        self.n_params = n_params

        def _body(*args):
            operands = list(args)
            if partition_name is not None:
                operands.append(bass2jax.partition_id_tensor())
            outs = bass2jax._bass_exec_p.bind(
                *operands,
                out_avals=tuple(out_avals),
                in_names=tuple(all_in_names),
                out_names=tuple(out_names),
                lowering_input_output_aliases=(),
                sim_require_finite=False,
                sim_require_nnan=False,
                nc=nc,
            )
            return tuple(outs)

        devices = jax.devices()[:n_cores]
        assert len(devices) == n_cores
        if n_cores == 1:
            self._fn = jax.jit(_body, keep_unused=True)
        else:
            mesh = Mesh(np.asarray(devices), ("core",))
            in_specs = (PartitionSpec("core"),) * (n_params + n_outs)
            out_specs = (PartitionSpec("core"),) * n_outs
            self._fn = jax.jit(
                shard_map(_body, mesh=mesh, in_specs=in_specs,
                          out_specs=out_specs, check_rep=False),
                keep_unused=True,
            )

    def prep_inputs(self, in_maps):
        per_core = [[np.asarray(m[n]) for n in self.in_names] for m in in_maps]
        if self.n_cores == 1:
            args = per_core[0]
        else:
            args = [
                np.concatenate([per_core[c][i] for c in range(self.n_cores)], 0)
                for i in range(self.n_params)
            ]
        zouts = []
        for (s, d) in self.out_shapes:
            full = (s[0] * self.n_cores,) + tuple(s[1:]) \
                if self.n_cores > 1 else s
            zouts.append(np.zeros(full, d))
        return args + zouts

    def run(self, args):
        import jax
        outs = self._fn(*args)
        jax.block_until_ready(outs)
        return outs
